# revision 2
# baseline (speedup 1.0000x reference)
"""ACM-GNN (3-branch GCN) distributed Bass kernel for one TRN2 chip (8 NeuronCores).

Contract: kernel(**inputs) takes FULL unsharded inputs (as produced by the
problem's setup_inputs) and returns the FULL [N, DOUT] float32 output of

    out = log_softmax(a_h*H_hp + a_l*H_lp + a_i*H_i, axis=1)

with H_hp = relu(xW_hp + b - Ahat(xW_hp + b)), H_lp = relu(Ahat(xW_lp + b)),
H_i = relu(xW_i + b), Ahat = D^-1/2 (A + I) D^-1/2, and per-node sigmoid gates.

Strategy (sharding_hint: partition nodes, replicate weights, exchange halos):
  * nodes are sharded 12500/core; within a core nodes are processed in
    in-degree-sorted order (host-side permutation) so that the per-tile gather
    grids have minimal padding; the host un-permutes the final output.
  * each core computes xW for its nodes (PE), builds pre-scaled rows
    g = [dinv*xw_hp | dinv*xw_lp]  (so the per-edge GCN norm factorizes into
    per-node scales), AllGathers the g-table, then pulls the rows of its
    in-edges with indirect DMAs (128 rows per call, one per dst-slot column)
    and segment-sums them with strided DVE reduces.  Self-loop terms are
    applied locally; all remaining per-node math (gates, log_softmax) is
    DVE/ACT work overlapped under the gather stream.
"""
import sys

sys.path.insert(0, "/opt/trn_rl_repo")

import numpy as np

import concourse.bacc as bacc
import concourse.bass as bass
import concourse.mybir as mybir
import concourse.tile as tile

NCORES = 8
N = 100000
E = 1600000
DIN = 256
DOUT = 64
NLOC = N // NCORES  # 12500
TILES = (NLOC + 127) // 128  # 98
NP = TILES * 128  # 12544 padded rows per core
DT = mybir.dt.float32


# ----------------------------------------------------------------- host prep
def _preprocess(x, edge_index):
    s = np.asarray(edge_index[0], dtype=np.int64)
    d = np.asarray(edge_index[1], dtype=np.int64)
    deg = np.bincount(d, minlength=N).astype(np.float64) + 1.0  # + self loop
    dinv = (1.0 / np.sqrt(deg)).astype(np.float32)
    indeg = np.bincount(d, minlength=N)  # edge-only in-degree (slot counts)

    # per-core permutation: sort local nodes by descending in-degree
    perm = np.empty((NCORES, NLOC), dtype=np.int64)
    pos = np.empty(N, dtype=np.int64)  # local slot of each node
    for c in range(NCORES):
        vs = np.arange(c * NLOC, (c + 1) * NLOC)
        order = np.argsort(-indeg[vs], kind="stable")
        p = vs[order]
        perm[c] = p
        pos[p] = np.arange(NLOC)

    # common per-tile column counts K[t] (max in-deg within tile over all cores)
    cnt = np.zeros((NCORES, NP), dtype=np.int64)
    loc_all = pos[np.arange(N)]
    cnt[np.arange(N) // NLOC, loc_all] = indeg
    K = cnt.reshape(NCORES, TILES, 128).max(axis=(0, 2)).astype(np.int64)
    K = np.maximum(K, 1)
    off = np.zeros(TILES + 1, dtype=np.int64)
    off[1:] = np.cumsum(128 * K)
    TOT = int(off[-1])

    # per-core gather index arrays (padded-global row ids, ZROW for pads)
    d_core = d // NLOC
    d_slot = pos[d]
    gsl = d_core * NLOC + d_slot  # global sort key
    order = np.argsort(gsl, kind="stable")
    so, ss = gsl[order], s[order]
    grp = np.bincount(so, minlength=NCORES * NLOC)
    starts = np.zeros(NCORES * NLOC, dtype=np.int64)
    starts[1:] = np.cumsum(grp)[:-1]
    j = np.arange(E) - np.repeat(starts[grp > 0], grp[grp > 0])
    gid_src = (ss // NLOC) * NP + pos[ss]  # padded-global row of each edge src
    dslot_o = so % NLOC
    dcore_o = so // NLOC
    tt = dslot_o // 128
    pp = dslot_o % 128
    posn = off[tt] + pp * K[tt] + j  # position inside that core's gidx
    gidx = np.empty((NCORES, TOT), dtype=np.int32)
    for c in range(NCORES):
        gidx[c] = c * NP + NLOC  # ZROW: first zero pad row of this core
        m = dcore_o == c
        gidx[c][posn[m]] = gid_src[m]

    # per-core inputs
    xTs, dinvs = [], []
    for c in range(NCORES):
        xp = np.zeros((NP, DIN), dtype=np.float32)
        xp[:NLOC] = np.asarray(x)[perm[c]]
        xTs.append(np.ascontiguousarray(xp.T))  # [DIN, NP]
        dv = np.zeros(NP, dtype=np.float32)
        dv[:NLOC] = dinv[perm[c]]
        dinvs.append(
            np.ascontiguousarray(dv.reshape(TILES, 128).T).ravel()
        )  # p-major [128*TILES]
    return perm, K, off, TOT, gidx, xTs, dinvs


# ------------------------------------------------------------- device kernel
def build_program(K, TOT):
    T = len(K)
    Kmax = int(max(K))
    nc = bacc.Bacc("TRN2", debug=False, num_devices=NCORES)
    xT = nc.dram_tensor("xT", [DIN, NP], DT, kind="ExternalInput")
    dinv_in = nc.dram_tensor("dinv", [128 * T], DT, kind="ExternalInput")
    wcat = nc.dram_tensor("wcat", [DIN, 192], DT, kind="ExternalInput")
    brep = nc.dram_tensor("brep", [128, 192], DT, kind="ExternalInput")
    grep = nc.dram_tensor("grep", [128, 192], DT, kind="ExternalInput")
    gb = nc.dram_tensor("gb", [128, 4], DT, kind="ExternalInput")
    gidx_in = nc.dram_tensor("gidx", [TOT], mybir.dt.int32, kind="ExternalInput")
    out = nc.dram_tensor("out", [NP, DOUT], DT, kind="ExternalOutput")

    AF = mybir.ActivationFunctionType
    OP = mybir.AluOpType

    with tile.TileContext(nc) as tc:
        with (
            tc.tile_pool(name="const", bufs=1) as cp,
            tc.tile_pool(name="persist", bufs=1) as pp,
            tc.tile_pool(name="dram", bufs=1, space="DRAM") as dp,
        ):
            g_loc = dp.tile([NP, 128], DT)
            g_tab = dp.tile([NCORES * NP, 128], DT, addr_space="Shared")

            wc = cp.tile([128, 2, 192], DT)
            nc.sync.dma_start(
                out=wc[:], in_=wcat[:].rearrange("(k p) f -> p k f", p=128)
            )
            br = cp.tile([128, 192], DT)
            nc.sync.dma_start(out=br[:], in_=brep[:])
            gr = cp.tile([128, 192], DT)
            nc.sync.dma_start(out=gr[:], in_=grep[:])
            gbt = cp.tile([128, 4], DT)
            nc.sync.dma_start(out=gbt[:], in_=gb[:])
            dv_sb = cp.tile([128, T], DT)
            nc.sync.dma_start(
                out=dv_sb[:], in_=dinv_in[:].rearrange("(p t) -> p t", p=128)
            )

            xw_hp = pp.tile([128, T * 64], DT)
            xw_lp = pp.tile([128, T * 64], DT)
            h_i = pp.tile([128, T * 64], DT)

            # ---------------- phase A: xW matmuls, g rows ----------------
            GRP = 14  # tiles per x-load group
            with (
                tc.tile_pool(name="xload", bufs=2) as xp,
                tc.tile_pool(name="psum", bufs=4, space="PSUM") as psp,
                tc.tile_pool(name="gout", bufs=3) as gop,
            ):
                for g0 in range(0, T, GRP):
                    ntg = min(GRP, T - g0)
                    xt = xp.tile([128, 2, GRP * 128], DT, name="xt", tag="xt")
                    nc.sync.dma_start(
                        out=xt[:, :, : ntg * 128],
                        in_=xT[:].rearrange("(k p) n -> p k n", p=128)[
                            :, :, g0 * 128 : (g0 + ntg) * 128
                        ],
                    )
                    for lt in range(ntg):
                        t = g0 + lt
                        ps = psp.tile([128, 192], DT, name="ps", tag="ps")
                        for k in range(2):
                            nc.tensor.matmul(
                                out=ps[:],
                                lhsT=xt[:, k, lt * 128 : (lt + 1) * 128],
                                rhs=wc[:, k, :],
                                start=(k == 0),
                                stop=(k == 1),
                            )
                        sl = slice(t * 64, (t + 1) * 64)
                        nc.vector.tensor_add(
                            out=xw_hp[:, sl], in0=ps[:, 0:64], in1=br[:, 0:64]
                        )
                        nc.vector.tensor_add(
                            out=xw_lp[:, sl], in0=ps[:, 64:128], in1=br[:, 64:128]
                        )
                        hi0 = gop.tile([128, 64], DT, name="hi0", tag="hi0")
                        nc.vector.tensor_add(
                            out=hi0[:], in0=ps[:, 128:192], in1=br[:, 128:192]
                        )
                        nc.scalar.activation(out=h_i[:, sl], in_=hi0[:], func=AF.Relu)
                        gt = gop.tile([128, 128], DT, name="gt", tag="gt")
                        nc.scalar.activation(
                            out=gt[:, 0:64], in_=xw_hp[:, sl], func=AF.Copy,
                            scale=dv_sb[:, t : t + 1],
                        )
                        nc.scalar.activation(
                            out=gt[:, 64:128], in_=xw_lp[:, sl], func=AF.Copy,
                            scale=dv_sb[:, t : t + 1],
                        )
                        nc.sync.dma_start(
                            out=g_loc[t * 128 : (t + 1) * 128, :], in_=gt[:]
                        )

            # ---------------- phase B: replicate the table ----------------
            nc.gpsimd.collective_compute(
                "AllGather",
                OP.bypass,
                replica_groups=[list(range(NCORES))],
                ins=[g_loc[:]],
                outs=[g_tab[:]],
            )

            # ------------- phase C: gather + reduce + node math -------------
            off = np.zeros(T + 1, dtype=np.int64)
            off[1:] = np.cumsum(128 * np.asarray(K))
            with (
                tc.tile_pool(name="idxp", bufs=3) as ip,
                tc.tile_pool(name="gath", bufs=2) as gp,
                tc.tile_pool(name="work", bufs=3) as wk,
            ):
                for t in range(T):
                    Kt = int(K[t])
                    idx_t = ip.tile([128, Kmax], mybir.dt.int32, name="idx", tag="idx")
                    nc.sync.dma_start(
                        out=idx_t[:, :Kt],
                        in_=gidx_in[int(off[t]) : int(off[t + 1])].rearrange(
                            "(p j) -> p j", p=128
                        ),
                    )
                    gt = gp.tile([128, Kmax * 128], DT, name="gbuf", tag="gbuf")
                    for jj in range(Kt):
                        nc.gpsimd.indirect_dma_start(
                            out=gt[:, jj * 128 : (jj + 1) * 128],
                            out_offset=None,
                            in_=g_tab[:],
                            in_offset=bass.IndirectOffsetOnAxis(
                                ap=idx_t[:, jj : jj + 1], axis=0
                            ),
                        )
                    acc = wk.tile([128, 128], DT, name="acc", tag="acc")
                    if Kt > 1:
                        nc.vector.tensor_reduce(
                            out=acc[:],
                            in_=gt[:, : Kt * 128].rearrange(
                                "p (k f) -> p f k", k=Kt
                            ),
                            axis=mybir.AxisListType.X,
                            op=OP.add,
                        )
                    else:
                        nc.vector.tensor_copy(out=acc[:], in_=gt[:, 0:128])

                    sl = slice(t * 64, (t + 1) * 64)
                    dv = dv_sb[:, t : t + 1]
                    # high-pass: H_hp = relu(xw_hp - dinv*(acc_hp + dinv*xw_hp))
                    t1 = wk.tile([128, 64], DT, name="t1", tag="t1")
                    nc.scalar.activation(out=t1[:], in_=xw_hp[:, sl], func=AF.Copy, scale=dv)
                    t2 = wk.tile([128, 64], DT, name="t2", tag="t2")
                    nc.vector.tensor_add(out=t2[:], in0=acc[:, 0:64], in1=t1[:])
                    p1 = wk.tile([128, 64], DT, name="p1", tag="p1")
                    nc.scalar.activation(out=p1[:], in_=t2[:], func=AF.Copy, scale=dv)
                    hhp0 = wk.tile([128, 64], DT, name="hhp0", tag="hhp0")
                    nc.vector.tensor_tensor(
                        out=hhp0[:], in0=xw_hp[:, sl], in1=p1[:], op=OP.subtract
                    )
                    hhp = wk.tile([128, 64], DT, name="hhp", tag="hhp")
                    nc.scalar.activation(out=hhp[:], in_=hhp0[:], func=AF.Relu)
                    # low-pass: H_lp = relu(dinv*(acc_lp + dinv*xw_lp))
                    t1l = wk.tile([128, 64], DT, name="t1l", tag="t1l")
                    nc.scalar.activation(out=t1l[:], in_=xw_lp[:, sl], func=AF.Copy, scale=dv)
                    t2l = wk.tile([128, 64], DT, name="t2l", tag="t2l")
                    nc.vector.tensor_add(out=t2l[:], in0=acc[:, 64:128], in1=t1l[:])
                    hlp = wk.tile([128, 64], DT, name="hlp", tag="hlp")
                    nc.scalar.activation(out=hlp[:], in_=t2l[:], func=AF.Relu, scale=dv)

                    # gates: a_b = sigmoid(sum_f H*w + bias)
                    ga = wk.tile([128, 4], DT, name="ga", tag="ga")
                    junk = wk.tile([128, 64], DT, name="junk", tag="junk")
                    sg = wk.tile([128, 4], DT, name="sg", tag="sg")
                    for b, h in ((0, hhp[:]), (1, hlp[:]), (2, h_i[:, sl])):
                        nc.vector.tensor_tensor(
                            out=junk[:], in0=h, in1=gr[:, b * 64 : b * 64 + 64],
                            op=OP.mult,
                        )
                        nc.vector.tensor_reduce(
                            out=ga[:, b : b + 1], in_=junk[:],
                            axis=mybir.AxisListType.X, op=OP.add,
                        )
                        nc.scalar.activation(
                            out=sg[:, b : b + 1], in_=ga[:, b : b + 1],
                            func=AF.Sigmoid, bias=gbt[:, b : b + 1],
                        )

                    # combine
                    o1 = wk.tile([128, 64], DT, name="o1", tag="o1")
                    nc.scalar.activation(out=o1[:], in_=hhp[:], func=AF.Copy, scale=sg[:, 0:1])
                    o2 = wk.tile([128, 64], DT, name="o2", tag="o2")
                    nc.scalar.activation(out=o2[:], in_=hlp[:], func=AF.Copy, scale=sg[:, 1:2])
                    o3 = wk.tile([128, 64], DT, name="o3", tag="o3")
                    nc.scalar.activation(out=o3[:], in_=h_i[:, sl], func=AF.Copy, scale=sg[:, 2:3])
                    s12 = wk.tile([128, 64], DT, name="s12", tag="s12")
                    nc.vector.tensor_add(out=s12[:], in0=o1[:], in1=o2[:])
                    sall = wk.tile([128, 64], DT, name="sall", tag="sall")
                    nc.vector.tensor_add(out=sall[:], in0=s12[:], in1=o3[:])

                    # log_softmax
                    nm = wk.tile([128, 1], DT, name="nm", tag="nm")
                    nc.vector.tensor_reduce(
                        out=nm[:], in_=sall[:], axis=mybir.AxisListType.X,
                        op=OP.max, negate=True,
                    )
                    et = wk.tile([128, 64], DT, name="et", tag="et")
                    zz = wk.tile([128, 1], DT, name="zz", tag="zz")
                    nc.scalar.activation(
                        out=et[:], in_=sall[:], func=AF.Exp, bias=nm[:, 0:1],
                        accum_out=zz[:],
                    )
                    lz = wk.tile([128, 1], DT, name="lz", tag="lz")
                    nc.scalar.activation(out=lz[:], in_=zz[:], func=AF.Ln)
                    nl = wk.tile([128, 1], DT, name="nl", tag="nl")
                    nc.vector.tensor_tensor(
                        out=nl[:], in0=nm[:], in1=lz[:], op=OP.subtract
                    )
                    res = wk.tile([128, 64], DT, name="res", tag="res")
                    nc.vector.tensor_scalar(
                        out=res[:], in0=sall[:], scalar1=nl[:, 0:1], scalar2=None,
                        op0=OP.add,
                    )
                    nc.sync.dma_start(
                        out=out[t * 128 : (t + 1) * 128, :], in_=res[:]
                    )
    nc.compile()
    return nc


# ----------------------------------------------------------------- execution
class _Runner:
    """jit(shard_map(bass_exec)) once; reusable timed executions."""

    def __init__(self, nc, n_cores=NCORES):
        import jax
        from jax.sharding import Mesh, PartitionSpec

        from concourse import bass2jax
        from concourse.bass2jax import _bass_exec_p, install_neuronx_cc_hook

        install_neuronx_cc_hook()
        self.jax = jax
        self.n_cores = n_cores
        partition_name = (
            nc.partition_id_tensor.name if nc.partition_id_tensor else None
        )
        in_names, out_names, out_avals = [], [], []
        for alloc in nc.m.functions[0].allocations:
            if not isinstance(alloc, mybir.MemoryLocationSet):
                continue
            name = alloc.memorylocations[0].name
            if alloc.kind == "ExternalInput":
                if name != partition_name:
                    in_names.append(name)
            elif alloc.kind == "ExternalOutput":
                out_names.append(name)
                out_avals.append(
                    jax.core.ShapedArray(
                        tuple(alloc.tensor_shape), mybir.dt.np(alloc.dtype)
                    )
                )
        self.in_names, self.out_names, self.out_avals = in_names, out_names, out_avals
        all_in_names = in_names + out_names
        if partition_name is not None:
            all_in_names.append(partition_name)

        def _body(*args):
            operands = list(args)
            if partition_name is not None:
                operands.append(bass2jax.partition_id_tensor())
            return tuple(
                _bass_exec_p.bind(
                    *operands,
                    out_avals=tuple(out_avals),
                    in_names=tuple(all_in_names),
                    out_names=tuple(out_names),
                    lowering_input_output_aliases=(),
                    sim_require_finite=True,
                    sim_require_nnan=True,
                    nc=nc,
                )
            )

        devices = jax.devices()[:n_cores]
        self.mesh = Mesh(np.asarray(devices), ("core",))
        n_io = len(in_names) + len(out_names)
        self.fn = jax.jit(
            jax.shard_map(
                _body,
                mesh=self.mesh,
                in_specs=(PartitionSpec("core"),) * n_io,
                out_specs=(PartitionSpec("core"),) * len(out_names),
                check_vma=False,
            ),
            keep_unused=True,
        )

    def load_inputs(self, in_maps):
        from jax.sharding import NamedSharding, PartitionSpec

        concat = [
            np.concatenate(
                [np.asarray(in_maps[c][n]) for c in range(self.n_cores)], axis=0
            )
            for n in self.in_names
        ]
        zeros = [
            np.zeros((self.n_cores * a.shape[0], *a.shape[1:]), a.dtype)
            for a in self.out_avals
        ]
        sh = NamedSharding(self.mesh, PartitionSpec("core"))
        self._dev_in = [self.jax.device_put(v, sh) for v in concat + zeros]
        return self

    def run(self):
        outs = self.fn(*self._dev_in)
        self.jax.block_until_ready(outs)
        return [
            {
                n: np.asarray(outs[i]).reshape(self.n_cores, *self.out_avals[i].shape)[c]
                for i, n in enumerate(self.out_names)
            }
            for c in range(self.n_cores)
        ]

    def time_wall_ns(self, iters=8, warmup=2):
        import time

        for _ in range(warmup):
            self.jax.block_until_ready(self.fn(*self._dev_in))
        ts = []
        for _ in range(iters):
            t0 = time.perf_counter_ns()
            self.jax.block_until_ready(self.fn(*self._dev_in))
            ts.append(time.perf_counter_ns() - t0)
        return min(ts)


_CACHE = {}


def _get_program_and_prep(x, edge_index):
    perm, K, off, TOT, gidx, xTs, dinvs = _preprocess(x, edge_index)
    key = (TOT, tuple(int(k) for k in K))
    if key not in _CACHE:
        _CACHE[key] = build_program(K, TOT)
    return _CACHE[key], perm, K, TOT, gidx, xTs, dinvs


def _make_in_maps(perm, gidx, xTs, dinvs, W_hp, b_hp, W_lp, b_lp, W_i, b_i,
                  w_h, b_h, w_l, b_l, w_i2, b_i2):
    wcat = np.concatenate(
        [np.asarray(W_hp), np.asarray(W_lp), np.asarray(W_i)], axis=1
    ).astype(np.float32)
    bcat = np.concatenate(
        [np.asarray(b_hp), np.asarray(b_lp), np.asarray(b_i)]
    ).astype(np.float32)
    brep = np.tile(bcat[None, :], (128, 1))
    gcat = np.concatenate(
        [np.asarray(w_h)[:, 0], np.asarray(w_l)[:, 0], np.asarray(w_i2)[:, 0]]
    ).astype(np.float32)
    grep = np.tile(gcat[None, :], (128, 1))
    gbv = np.array(
        [float(np.asarray(b_h)[0]), float(np.asarray(b_l)[0]),
         float(np.asarray(b_i2)[0]), 0.0], dtype=np.float32,
    )
    gb = np.tile(gbv[None, :], (128, 1))
    return [
        {
            "xT": xTs[c],
            "dinv": dinvs[c],
            "wcat": wcat,
            "brep": brep,
            "grep": grep,
            "gb": gb,
            "gidx": gidx[c],
        }
        for c in range(NCORES)
    ]


def kernel(x, edge_index, W_hp, b_hp, W_lp, b_lp, W_i, b_i,
           w_h, b_h, w_l, b_l, w_i2, b_i2):
    nc, perm, K, TOT, gidx, xTs, dinvs = _get_program_and_prep(x, edge_index)
    in_maps = _make_in_maps(perm, gidx, xTs, dinvs, W_hp, b_hp, W_lp, b_lp,
                            W_i, b_i, w_h, b_h, w_l, b_l, w_i2, b_i2)
    r = _Runner(nc).load_inputs(in_maps)
    res = r.run()
    out = np.empty((N, DOUT), dtype=np.float32)
    for c in range(NCORES):
        out[perm[c]] = res[c]["out"][:NLOC]
    kernel._last_runner = r  # for test harness timing
    return out
